# revision 27
# baseline (speedup 1.0000x reference)
"""Trainium2 Bass kernel for nn_BaseAttention (causal MHA, b=2, n=2048, d=1024, 16 heads).

Sharding (8 cores): core c handles batch c//4 and heads 4*(c%4)..4*(c%4)+3.
- W_q/W_k/W_v column-sharded (256 cols/core), W_o row-sharded (256 rows/core).
- Each core computes a partial output [2048, 1024] in fp32; host sums the 4
  partials per batch (row-parallel out-projection) and stacks the 2 batches.

Per-core kernel (bf16 data path, fp32 PSUM accumulation; ~160 us HW exec,
rel err ~3.5e-3 vs the fp32 reference):
  - x is transposed + bf16-cast on the host (shared by the 4 cores of each
    batch) so x^T loads as full-bandwidth contiguous copy DMAs; weights are
    pre-laid-out to their SBUF layouts on the host as well.
  - Q^T/K^T projections emitted transposed; V natural with a ones column per
    head ([V|1] trick: the AV matmul yields ctx^T on psum partitions 0..63
    and the softmax row-sum at partition 64 in one pass).
  - attention per (head-pair, q-tile j): S^T = K_h @ Q_h^T on PE (even/odd
    heads on disjoint PE row-halves), exp on ACT over [128,1024] psum pairs
    with the 1/8 scale fused, causal mask via gpsimd affine_select
    (exp-then-zero; S/AV/mask all narrowed to the valid causal q-range), AV
    pipelined one i-pair behind S, normalization via DVE
    reciprocal_approx_fast (SBUF input only - broken from PSUM) + gpsimd
    partition_broadcast + DVE multiply.
  - projection work of round g+1 (or out-projection chunks in the last
    round) is woven between attention steps so the in-order PE queue never
    idles while ACT catches up (keeps HAM warm).
  - out-projection from ctx^T; bias added by a DVE tensor_add against a
    pre-broadcast b_o tile during the PSUM drain.
  - copy-DMAs and transpose-DMAs must not interleave (xbar mode transitions
    serialize); all DMAs here are copies. DMA issue is ~0.6us each on a
    sequencer, so transfers are consolidated into few instructions and
    split across the sync + scalar HWDGE queues.
"""
import sys, types

sys.path.insert(0, "/opt/trn_rl_repo")


def _install_ntff_shim():
    # antenv.axon_hooks is absent in this image; register the NTFF profile
    # hook via ctypes so run_bass_kernel_spmd(trace=True) works under axon.
    if "antenv.axon_hooks" in sys.modules:
        return
    try:
        sys.path.insert(0, "/root/.axon_site")
        from trn_agent_boot.trn_boot import _ntff_profile_via_ctypes

        hook = _ntff_profile_via_ctypes("/opt/axon/libaxon_pjrt.so")
        mod = types.ModuleType("antenv.axon_hooks")
        mod.get_axon_ntff_profile_hook = lambda: hook
        mod.set_axon_ntff_profile_hook = lambda h: None
        sys.modules["antenv.axon_hooks"] = mod
    except Exception:
        pass


_install_ntff_shim()

import numpy as np
import ml_dtypes
import concourse.bass as bass
import concourse.mybir as mybir
import concourse.tile as tile
from concourse import bacc
from concourse.bass_utils import run_bass_kernel_spmd
from contextlib import ExitStack

f32 = mybir.dt.float32
bf16 = mybir.dt.bfloat16
EXP = mybir.ActivationFunctionType.Exp

SEQ = 2048          # sequence length
DIN = 1024          # model dim (8 chunks of 128)
QC = 256            # q/k/v cols per core (4 heads x 64)
HD = 64             # head dim
NH = 4              # heads per core
NG = 4              # row groups of 512
VST = NH * 65       # Vones stride per row chunk (4 heads x (64 V + 1 ones))

TRACE = False
LAST_RESULTS = None


def build_nc():
    nc = bacc.Bacc()
    x_d = nc.dram_tensor("x", [DIN, SEQ], bf16, kind="ExternalInput")  # pre-transposed on host
    wq_d = nc.dram_tensor("wq", [128, 8 * QC], bf16, kind="ExternalInput")
    wk_d = nc.dram_tensor("wk", [128, 8 * QC], bf16, kind="ExternalInput")
    wv_d = nc.dram_tensor("wv", [128, 8 * QC], bf16, kind="ExternalInput")
    wo_d = nc.dram_tensor("wo", [128, 2 * DIN], bf16, kind="ExternalInput")
    bo_d = nc.dram_tensor("bo", [1, DIN], bf16, kind="ExternalInput")
    out_d = nc.dram_tensor("out", [SEQ, DIN], f32, kind="ExternalOutput")

    with tile.TileContext(nc, pool_alloc_mode="queue") as tc, ExitStack() as ctx:
        cst = ctx.enter_context(tc.tile_pool(name="cst", bufs=1))
        wr = ctx.enter_context(tc.tile_pool(name="wr", bufs=1))
        big = ctx.enter_context(tc.tile_pool(name="big", bufs=1))
        ptp = ctx.enter_context(tc.tile_pool(name="ptp", bufs=8))
        nrm = ctx.enter_context(tc.tile_pool(name="nrm", bufs=3))
        ob = ctx.enter_context(tc.tile_pool(name="ob", bufs=6))
        ps = ctx.enter_context(tc.tile_pool(name="ps", bufs=1, space="PSUM"))

        # ---- DMAs, ordered by first use (weights pre-laid-out on host).
        # weights issue on the scalar HWDGE queue, x on sync: parallel issue.
        def dma_w(wd, n_inner):
            name = wd.name + "_sb"
            t = wr.tile([128, 8 * n_inner], bf16, name=name)
            nc.scalar.dma_start(t[:], wd[:])
            return t

        # tiny bias DMA first so the gpsimd bias broadcast clears the FIFO
        # before any affine_select masks queue behind it
        bo_sb = cst.tile([1, DIN], bf16)
        nc.scalar.dma_start(bo_sb[:], bo_d[:])
        bo_f = cst.tile([1, DIN], f32)
        nc.vector.tensor_copy(bo_f[:], bo_sb[:])
        bias_bc = cst.tile([128, DIN], f32)
        nc.gpsimd.partition_broadcast(bias_bc[:], bo_f[:])

        # x arrives pre-transposed: straight contiguous copy DMAs at full BW.
        # xT[p, c*2048+r] = x[r, c*128+p] = x_d[c*128+p, r]
        wq_sb = wr.tile([128, 8 * QC], bf16, name="wq_sb")
        nc.scalar.dma_start(wq_sb[:, 0:2 * QC], wq_d[:, 0:2 * QC])
        nc.scalar.dma_start(wq_sb[:, 2 * QC:], wq_d[:, 2 * QC:])
        xT = big.tile([128, 8 * SEQ], bf16)
        xview = xT[:].rearrange("p (c r) -> p c r", r=SEQ)
        dview = x_d.rearrange("(c p) r -> p c r", p=128)
        nc.sync.dma_start(xview[:, 0:2, 0:512], dview[:, 0:2, 0:512])
        nc.sync.dma_start(xview[:, 2:8, 0:512], dview[:, 2:8, 0:512])
        wk_sb = dma_w(wk_d, QC)
        wv_sb = dma_w(wv_d, QC)
        for g in range(1, NG):
            nc.sync.dma_start(
                xview[:, :, g * 512:(g + 1) * 512],
                dview[:, :, g * 512:(g + 1) * 512],
            )
        wo_sb = cst.tile([128, 2 * DIN], bf16)
        nc.scalar.dma_start(wo_sb[:], wo_d[:])

        # ---- persistent activations ----
        qt_sb = [big.tile([128, SEQ], bf16, name=f"qt{t}") for t in range(2)]
        kt_sb = [big.tile([128, SEQ], bf16, name=f"kt{t}") for t in range(2)]
        vones = big.tile([128, 16 * VST], bf16)
        ctxt = [big.tile([128, SEQ], bf16, name=f"ctxt{t}") for t in range(2)]

        vview = vones.rearrange("p (r h e) -> p r h e", h=NH, e=65)
        nc.vector.memset(vview[:, :, :, 64], 1.0)

        # ---- emission helpers ----
        def emit_qk(g, t, wt, dst):
            prj = ps.tile([128, 512], f32, tag="b", bufs=2, name="prj")
            for c in range(8):
                nc.tensor.matmul(
                    prj[:],
                    wt[:, c * QC + t * 128: c * QC + t * 128 + 128],
                    xT[:, c * SEQ + g * 512: c * SEQ + g * 512 + 512],
                    start=(c == 0),
                    stop=(c == 7),
                )
            nc.vector.tensor_copy(dst[t][:, g * 512:(g + 1) * 512], prj[:])

        def emit_v(g, rc):
            rcg = 4 * g + rc
            vps = ps.tile([128, 256], f32, tag="b", bufs=2, name="vps")
            for c in range(8):
                nc.tensor.matmul(
                    vps[:],
                    xT[:, c * SEQ + rcg * 128: c * SEQ + rcg * 128 + 128],
                    wv_sb[:, c * QC:(c + 1) * QC],
                    start=(c == 0),
                    stop=(c == 7),
                )
            nc.vector.tensor_copy(
                vview[:, rcg, :, 0:64],
                vps[:].rearrange("p (h e) -> p h e", e=HD),
            )

        def emit_outproj(rc, n, tag="b"):
            ops = ps.tile([128, 512], f32, tag=tag, bufs=2, name="ops")
            for u in range(2):
                nc.tensor.matmul(
                    ops[:],
                    ctxt[u][:, rc * 128:(rc + 1) * 128],
                    wo_sb[:, u * DIN + n * 512: u * DIN + n * 512 + 512],
                    start=(u == 0),
                    stop=(u == 1),
                )
            osb = ob.tile([128, 512], f32, tag="o", name="osb")
            nc.vector.tensor_add(osb[:], ops[:], bias_bc[:, n * 512:(n + 1) * 512])
            nc.sync.dma_start(
                out_d[rc * 128:(rc + 1) * 128, n * 512:(n + 1) * 512], osb[:]
            )

        def proj_chunks(g):
            for t in range(2):
                yield lambda t=t: emit_qk(g, t, wq_sb, qt_sb)
                yield lambda t=t: emit_qk(g, t, wk_sb, kt_sb)
            for rc in range(4):
                yield lambda rc=rc: emit_v(g, rc)

        # ---- round 0 projections up-front ----
        for f in proj_chunks(0):
            f()

        # ---- main rounds: attention(j=g) woven with proj(g+1)/outproj ----
        for g in range(NG):
            j = g
            imax = 4 * j + 3
            npair = (imax + 1) // 2
            if g < NG - 1:
                filler = list(proj_chunks(g + 1))
            else:
                filler = [
                    (lambda rc=rc, n=n: emit_outproj(rc, n))
                    for rc in range(12)
                    for n in range(2)
                ]
            steps_total = 2 * npair
            fill_i = 0
            step = 0

            for u in range(2):           # head pair u: heads 2u, 2u+1
                avs = [ps.tile([65, 512], f32, tag="av", bufs=2, name=f"av{p}")
                       for p in range(2)]
                pts = [[], []]           # per parity: list of [128,1024] pair tiles
                for ip in range(npair):
                    i0 = 2 * ip
                    cur = []
                    for p in range(2):
                        sps = ps.tile([128, 1024], f32, tag="a", bufs=2, name="sps")
                        cur.append(sps)
                    for half in range(2):
                        i = i0 + half
                        off = max(0, 128 * i - 512 * j)
                        for p in range(2):
                            o = p * 64
                            nc.tensor.matmul(
                                cur[p][:, half * 512 + off:(half + 1) * 512],
                                kt_sb[u][o:o + 64, i * 128:(i + 1) * 128],
                                qt_sb[u][o:o + 64, j * 512 + off:(j + 1) * 512],
                                start=True,
                                stop=True,
                            )
                    for p in range(2):
                        pt = ptp.tile([128, 1024], bf16, tag="pt", name="pt")
                        nc.scalar.activation(pt[:], cur[p][:], EXP, scale=0.125)
                        for half in range(2):
                            i = i0 + half
                            if i >= 4 * j:
                                off = 128 * i - 512 * j
                                nc.gpsimd.affine_select(
                                    out=pt[:, half * 512 + off:(half + 1) * 512],
                                    in_=pt[:, half * 512 + off:(half + 1) * 512],
                                    compare_op=mybir.AluOpType.is_ge,
                                    fill=0.0,
                                    base=0,
                                    channel_multiplier=-1,
                                    pattern=[[1, 512 - off]],
                                )
                        pts[p].append(pt)
                    if ip >= 1:
                        kp = ip - 1
                        for p in range(2):
                            h = 2 * u + p
                            for half in range(2):
                                k = 2 * kp + half
                                off = max(0, 128 * k - 512 * j)
                                nc.tensor.matmul(
                                    avs[p][:, off:512],
                                    vones[:, k * VST + h * 65: k * VST + h * 65 + 65],
                                    pts[p][kp][:, half * 512 + off:(half + 1) * 512],
                                    start=(k == 0),
                                    stop=False,
                                )
                    step += 1
                    want = (len(filler) * step) // steps_total
                    while fill_i < want:
                        filler[fill_i]()
                        fill_i += 1
                # tail AVs for the last pair
                kp = npair - 1
                for p in range(2):
                    h = 2 * u + p
                    for half in range(2):
                        k = 2 * kp + half
                        off = max(0, 128 * k - 512 * j)
                        nc.tensor.matmul(
                            avs[p][:, off:512],
                            vones[:, k * VST + h * 65: k * VST + h * 65 + 65],
                            pts[p][kp][:, half * 512 + off:(half + 1) * 512],
                            start=(k == 0),
                            stop=(half == 1),
                        )
                # normalize: ctx^T = av[0:64] * (1/rowsum) broadcast
                for p in range(2):
                    o = p * 64
                    rsrow = nrm.tile([1, 512], f32, tag="rsrow", name="rsrow")
                    nc.vector.tensor_copy(rsrow[:], avs[p][64:65, :])
                    rinv = nrm.tile([1, 512], f32, tag="rinv", name="rinv")
                    nc.vector.reciprocal_approx_fast(rinv[:], rsrow[:])
                    bcast = nrm.tile([64, 512], f32, tag="bcast", name="bcast")
                    nc.gpsimd.partition_broadcast(bcast[:], rinv[:])
                    nc.vector.tensor_mul(
                        ctxt[u][o:o + 64, j * 512:(j + 1) * 512],
                        avs[p][0:64, :],
                        bcast[:],
                    )
            while fill_i < len(filler):
                filler[fill_i]()
                fill_i += 1

        # ---- final out-projection chunks (use idle av psum slots too) ----
        for rc in range(12, 16):
            for n in range(2):
                emit_outproj(rc, n, tag="av" if (rc + n) % 2 else "b")

    nc.compile()
    return nc


_NC = None


def _get_nc():
    global _NC
    if _NC is None:
        _NC = build_nc()
    return _NC


def kernel(x, W_q, W_k, W_v, W_o, b_o):
    global LAST_RESULTS
    nc = _get_nc()
    bf = ml_dtypes.bfloat16
    x = np.asarray(x, np.float32).astype(bf)
    # pre-transpose per batch (shared by the 4 cores of each batch)
    xT = [np.ascontiguousarray(x[bi].T) for bi in range(2)]
    W_q = np.asarray(W_q, np.float32).astype(bf)
    W_k = np.asarray(W_k, np.float32).astype(bf)
    W_v = np.asarray(W_v, np.float32).astype(bf)
    W_o = np.asarray(W_o, np.float32).astype(bf)
    b_o = np.asarray(b_o, np.float32).astype(bf).reshape(1, DIN)
    zeros_bo = np.zeros((1, DIN), bf)

    def lay_w(w, sl):   # [1024, 256] shard -> [128, 8*256]: t[p, c*256+n] = w[c*128+p, sl][n]
        return np.ascontiguousarray(
            w[:, sl].reshape(8, 128, QC).transpose(1, 0, 2).reshape(128, 8 * QC))

    def lay_wo(w, sl):  # [256, 1024] shard -> [128, 2*1024]
        return np.ascontiguousarray(
            w[sl, :].reshape(2, 128, DIN).transpose(1, 0, 2).reshape(128, 2 * DIN))

    in_maps = []
    for c in range(8):
        bi, g = c // 4, c % 4
        sl = slice(g * QC, (g + 1) * QC)
        in_maps.append({
            "x": xT[bi],
            "wq": lay_w(W_q, sl),
            "wk": lay_w(W_k, sl),
            "wv": lay_w(W_v, sl),
            "wo": lay_wo(W_o, sl),
            "bo": b_o if g == 0 else zeros_bo,
        })

    res = run_bass_kernel_spmd(nc, in_maps, list(range(8)), trace=TRACE)
    LAST_RESULTS = res
    outs = [np.asarray(r["out"], dtype=np.float32) for r in res.results]
    return np.stack([
        outs[0] + outs[1] + outs[2] + outs[3],
        outs[4] + outs[5] + outs[6] + outs[7],
    ])


if __name__ == "__main__":
    if "--compile-only" in sys.argv:
        import tempfile
        from concourse.bass_utils import compile_bass_kernel

        nc = build_nc()
        with tempfile.TemporaryDirectory() as td:
            print("walrus compiling...")
            neff = compile_bass_kernel(nc, td)
            print("COMPILE OK", neff)


# revision 28
# speedup vs baseline: 1.0017x; 1.0017x over previous
"""Trainium2 Bass kernel for nn_BaseAttention (causal MHA, b=2, n=2048, d=1024, 16 heads).

Sharding (8 cores): core c handles batch c//4 and heads 4*(c%4)..4*(c%4)+3.
- W_q/W_k/W_v column-sharded (256 cols/core), W_o row-sharded (256 rows/core).
- Each core computes a partial output [2048, 1024] in fp32; host sums the 4
  partials per batch (row-parallel out-projection) and stacks the 2 batches.

Per-core kernel (bf16 data path, fp32 PSUM accumulation; ~160 us HW exec,
rel err ~3.5e-3 vs the fp32 reference):
  - x is transposed + bf16-cast on the host (shared by the 4 cores of each
    batch) so x^T loads as full-bandwidth contiguous copy DMAs; weights are
    pre-laid-out to their SBUF layouts on the host as well.
  - Q^T/K^T projections emitted transposed; V natural with a ones column per
    head ([V|1] trick: the AV matmul yields ctx^T on psum partitions 0..63
    and the softmax row-sum at partition 64 in one pass).
  - attention per (head-pair, q-tile j): S^T = K_h @ Q_h^T on PE (even/odd
    heads on disjoint PE row-halves), exp on ACT over [128,1024] psum pairs
    with the 1/8 scale fused, causal mask via gpsimd affine_select
    (exp-then-zero; S/AV/mask all narrowed to the valid causal q-range), AV
    pipelined one i-pair behind S, normalization via DVE
    reciprocal_approx_fast (SBUF input only - broken from PSUM) + gpsimd
    partition_broadcast + DVE multiply.
  - projection work of round g+1 (or out-projection chunks in the last
    round) is woven between attention steps so the in-order PE queue never
    idles while ACT catches up (keeps HAM warm).
  - out-projection from ctx^T; bias added by a DVE tensor_add against a
    pre-broadcast b_o tile during the PSUM drain.
  - copy-DMAs and transpose-DMAs must not interleave (xbar mode transitions
    serialize); all DMAs here are copies. DMA issue is ~0.6us each on a
    sequencer, so transfers are consolidated into few instructions and
    split across the sync + scalar HWDGE queues.
"""
import sys, types

sys.path.insert(0, "/opt/trn_rl_repo")


def _install_ntff_shim():
    # antenv.axon_hooks is absent in this image; register the NTFF profile
    # hook via ctypes so run_bass_kernel_spmd(trace=True) works under axon.
    if "antenv.axon_hooks" in sys.modules:
        return
    try:
        sys.path.insert(0, "/root/.axon_site")
        from trn_agent_boot.trn_boot import _ntff_profile_via_ctypes

        hook = _ntff_profile_via_ctypes("/opt/axon/libaxon_pjrt.so")
        mod = types.ModuleType("antenv.axon_hooks")
        mod.get_axon_ntff_profile_hook = lambda: hook
        mod.set_axon_ntff_profile_hook = lambda h: None
        sys.modules["antenv.axon_hooks"] = mod
    except Exception:
        pass


_install_ntff_shim()

import numpy as np
import ml_dtypes
import concourse.bass as bass
import concourse.mybir as mybir
import concourse.tile as tile
from concourse import bacc
from concourse.bass_utils import run_bass_kernel_spmd
from contextlib import ExitStack

f32 = mybir.dt.float32
bf16 = mybir.dt.bfloat16
EXP = mybir.ActivationFunctionType.Exp

SEQ = 2048          # sequence length
DIN = 1024          # model dim (8 chunks of 128)
QC = 256            # q/k/v cols per core (4 heads x 64)
HD = 64             # head dim
NH = 4              # heads per core
NG = 4              # row groups of 512
VST = NH * 65       # Vones stride per row chunk (4 heads x (64 V + 1 ones))

TRACE = False
LAST_RESULTS = None


def build_nc():
    nc = bacc.Bacc()
    x_d = nc.dram_tensor("x", [DIN, SEQ], bf16, kind="ExternalInput")  # pre-transposed on host
    wq_d = nc.dram_tensor("wq", [128, 8 * QC], bf16, kind="ExternalInput")
    wk_d = nc.dram_tensor("wk", [128, 8 * QC], bf16, kind="ExternalInput")
    wv_d = nc.dram_tensor("wv", [128, 8 * QC], bf16, kind="ExternalInput")
    wo_d = nc.dram_tensor("wo", [128, 2 * DIN], bf16, kind="ExternalInput")
    bo_d = nc.dram_tensor("bo", [1, DIN], bf16, kind="ExternalInput")
    out_d = nc.dram_tensor("out", [SEQ, DIN], f32, kind="ExternalOutput")

    with tile.TileContext(nc, pool_alloc_mode="queue") as tc, ExitStack() as ctx:
        cst = ctx.enter_context(tc.tile_pool(name="cst", bufs=1))
        wr = ctx.enter_context(tc.tile_pool(name="wr", bufs=1))
        big = ctx.enter_context(tc.tile_pool(name="big", bufs=1))
        ptp = ctx.enter_context(tc.tile_pool(name="ptp", bufs=8))
        nrm = ctx.enter_context(tc.tile_pool(name="nrm", bufs=3))
        ob = ctx.enter_context(tc.tile_pool(name="ob", bufs=6))
        ps = ctx.enter_context(tc.tile_pool(name="ps", bufs=1, space="PSUM"))

        # ---- DMAs, ordered by first use (weights pre-laid-out on host).
        # weights issue on the scalar HWDGE queue, x on sync: parallel issue.
        def dma_w(wd, n_inner):
            name = wd.name + "_sb"
            t = wr.tile([128, 8 * n_inner], bf16, name=name)
            nc.scalar.dma_start(t[:], wd[:])
            return t

        # tiny bias DMA first so the gpsimd bias broadcast clears the FIFO
        # before any affine_select masks queue behind it
        bo_sb = cst.tile([1, DIN], bf16)
        nc.scalar.dma_start(bo_sb[:], bo_d[:])
        bo_f = cst.tile([1, DIN], f32)
        nc.vector.tensor_copy(bo_f[:], bo_sb[:])
        bias_bc = cst.tile([128, DIN], f32)
        nc.gpsimd.partition_broadcast(bias_bc[:], bo_f[:])

        # x arrives pre-transposed: straight contiguous copy DMAs at full BW.
        # xT[p, c*2048+r] = x[r, c*128+p] = x_d[c*128+p, r]
        wq_sb = wr.tile([128, 8 * QC], bf16, name="wq_sb")
        nc.scalar.dma_start(wq_sb[:, 0:2 * QC], wq_d[:, 0:2 * QC])
        nc.scalar.dma_start(wq_sb[:, 2 * QC:], wq_d[:, 2 * QC:])
        xT = big.tile([128, 8 * SEQ], bf16)
        xview = xT[:].rearrange("p (c r) -> p c r", r=SEQ)
        dview = x_d.rearrange("(c p) r -> p c r", p=128)
        nc.sync.dma_start(xview[:, 0:2, 0:512], dview[:, 0:2, 0:512])
        nc.sync.dma_start(xview[:, 2:8, 0:512], dview[:, 2:8, 0:512])
        wk_sb = dma_w(wk_d, QC)
        # g1 on the scalar queue (parallel with sync's g0 tail), g2/g3 on sync
        nc.scalar.dma_start(xview[:, :, 512:1024], dview[:, :, 512:1024])
        wv_sb = dma_w(wv_d, QC)
        for g in range(2, NG):
            nc.sync.dma_start(
                xview[:, :, g * 512:(g + 1) * 512],
                dview[:, :, g * 512:(g + 1) * 512],
            )
        wo_sb = cst.tile([128, 2 * DIN], bf16)
        nc.scalar.dma_start(wo_sb[:], wo_d[:])

        # ---- persistent activations ----
        qt_sb = [big.tile([128, SEQ], bf16, name=f"qt{t}") for t in range(2)]
        kt_sb = [big.tile([128, SEQ], bf16, name=f"kt{t}") for t in range(2)]
        vones = big.tile([128, 16 * VST], bf16)
        ctxt = [big.tile([128, SEQ], bf16, name=f"ctxt{t}") for t in range(2)]

        vview = vones.rearrange("p (r h e) -> p r h e", h=NH, e=65)
        nc.vector.memset(vview[:, :, :, 64], 1.0)

        # ---- emission helpers ----
        def emit_qk(g, t, wt, dst):
            prj = ps.tile([128, 512], f32, tag="b", bufs=2, name="prj")
            for c in range(8):
                nc.tensor.matmul(
                    prj[:],
                    wt[:, c * QC + t * 128: c * QC + t * 128 + 128],
                    xT[:, c * SEQ + g * 512: c * SEQ + g * 512 + 512],
                    start=(c == 0),
                    stop=(c == 7),
                )
            nc.vector.tensor_copy(dst[t][:, g * 512:(g + 1) * 512], prj[:])

        def emit_v(g, rc):
            rcg = 4 * g + rc
            vps = ps.tile([128, 256], f32, tag="b", bufs=2, name="vps")
            for c in range(8):
                nc.tensor.matmul(
                    vps[:],
                    xT[:, c * SEQ + rcg * 128: c * SEQ + rcg * 128 + 128],
                    wv_sb[:, c * QC:(c + 1) * QC],
                    start=(c == 0),
                    stop=(c == 7),
                )
            nc.vector.tensor_copy(
                vview[:, rcg, :, 0:64],
                vps[:].rearrange("p (h e) -> p h e", e=HD),
            )

        def emit_outproj(rc, n, tag="b"):
            ops = ps.tile([128, 512], f32, tag=tag, bufs=2, name="ops")
            for u in range(2):
                nc.tensor.matmul(
                    ops[:],
                    ctxt[u][:, rc * 128:(rc + 1) * 128],
                    wo_sb[:, u * DIN + n * 512: u * DIN + n * 512 + 512],
                    start=(u == 0),
                    stop=(u == 1),
                )
            osb = ob.tile([128, 512], f32, tag="o", name="osb")
            nc.vector.tensor_add(osb[:], ops[:], bias_bc[:, n * 512:(n + 1) * 512])
            nc.sync.dma_start(
                out_d[rc * 128:(rc + 1) * 128, n * 512:(n + 1) * 512], osb[:]
            )

        def proj_chunks(g):
            for t in range(2):
                yield lambda t=t: emit_qk(g, t, wq_sb, qt_sb)
                yield lambda t=t: emit_qk(g, t, wk_sb, kt_sb)
            for rc in range(4):
                yield lambda rc=rc: emit_v(g, rc)

        # ---- round 0 projections up-front ----
        for f in proj_chunks(0):
            f()

        # ---- main rounds: attention(j=g) woven with proj(g+1)/outproj ----
        for g in range(NG):
            j = g
            imax = 4 * j + 3
            npair = (imax + 1) // 2
            if g < NG - 1:
                filler = list(proj_chunks(g + 1))
            else:
                filler = [
                    (lambda rc=rc, n=n: emit_outproj(rc, n))
                    for rc in range(12)
                    for n in range(2)
                ]
            steps_total = 2 * npair
            fill_i = 0
            step = 0

            for u in range(2):           # head pair u: heads 2u, 2u+1
                avs = [ps.tile([65, 512], f32, tag="av", bufs=2, name=f"av{p}")
                       for p in range(2)]
                pts = [[], []]           # per parity: list of [128,1024] pair tiles
                for ip in range(npair):
                    i0 = 2 * ip
                    cur = []
                    for p in range(2):
                        sps = ps.tile([128, 1024], f32, tag="a", bufs=2, name="sps")
                        cur.append(sps)
                    for half in range(2):
                        i = i0 + half
                        off = max(0, 128 * i - 512 * j)
                        for p in range(2):
                            o = p * 64
                            nc.tensor.matmul(
                                cur[p][:, half * 512 + off:(half + 1) * 512],
                                kt_sb[u][o:o + 64, i * 128:(i + 1) * 128],
                                qt_sb[u][o:o + 64, j * 512 + off:(j + 1) * 512],
                                start=True,
                                stop=True,
                            )
                    for p in range(2):
                        pt = ptp.tile([128, 1024], bf16, tag="pt", name="pt")
                        nc.scalar.activation(pt[:], cur[p][:], EXP, scale=0.125)
                        for half in range(2):
                            i = i0 + half
                            if i >= 4 * j:
                                off = 128 * i - 512 * j
                                nc.gpsimd.affine_select(
                                    out=pt[:, half * 512 + off:(half + 1) * 512],
                                    in_=pt[:, half * 512 + off:(half + 1) * 512],
                                    compare_op=mybir.AluOpType.is_ge,
                                    fill=0.0,
                                    base=0,
                                    channel_multiplier=-1,
                                    pattern=[[1, 512 - off]],
                                )
                        pts[p].append(pt)
                    if ip >= 1:
                        kp = ip - 1
                        for p in range(2):
                            h = 2 * u + p
                            for half in range(2):
                                k = 2 * kp + half
                                off = max(0, 128 * k - 512 * j)
                                nc.tensor.matmul(
                                    avs[p][:, off:512],
                                    vones[:, k * VST + h * 65: k * VST + h * 65 + 65],
                                    pts[p][kp][:, half * 512 + off:(half + 1) * 512],
                                    start=(k == 0),
                                    stop=False,
                                )
                    step += 1
                    want = (len(filler) * step) // steps_total
                    while fill_i < want:
                        filler[fill_i]()
                        fill_i += 1
                # tail AVs for the last pair
                kp = npair - 1
                for p in range(2):
                    h = 2 * u + p
                    for half in range(2):
                        k = 2 * kp + half
                        off = max(0, 128 * k - 512 * j)
                        nc.tensor.matmul(
                            avs[p][:, off:512],
                            vones[:, k * VST + h * 65: k * VST + h * 65 + 65],
                            pts[p][kp][:, half * 512 + off:(half + 1) * 512],
                            start=(k == 0),
                            stop=(half == 1),
                        )
                # normalize: ctx^T = av[0:64] * (1/rowsum) broadcast
                for p in range(2):
                    o = p * 64
                    rsrow = nrm.tile([1, 512], f32, tag="rsrow", name="rsrow")
                    nc.vector.tensor_copy(rsrow[:], avs[p][64:65, :])
                    rinv = nrm.tile([1, 512], f32, tag="rinv", name="rinv")
                    nc.vector.reciprocal_approx_fast(rinv[:], rsrow[:])
                    bcast = nrm.tile([64, 512], f32, tag="bcast", name="bcast")
                    nc.gpsimd.partition_broadcast(bcast[:], rinv[:])
                    nc.vector.tensor_mul(
                        ctxt[u][o:o + 64, j * 512:(j + 1) * 512],
                        avs[p][0:64, :],
                        bcast[:],
                    )
            while fill_i < len(filler):
                filler[fill_i]()
                fill_i += 1

        # ---- final out-projection chunks (use idle av psum slots too) ----
        for rc in range(12, 16):
            for n in range(2):
                emit_outproj(rc, n, tag="av" if (rc + n) % 2 else "b")

    nc.compile()
    return nc


_NC = None


def _get_nc():
    global _NC
    if _NC is None:
        _NC = build_nc()
    return _NC


def kernel(x, W_q, W_k, W_v, W_o, b_o):
    global LAST_RESULTS
    nc = _get_nc()
    bf = ml_dtypes.bfloat16
    x = np.asarray(x, np.float32).astype(bf)
    # pre-transpose per batch (shared by the 4 cores of each batch)
    xT = [np.ascontiguousarray(x[bi].T) for bi in range(2)]
    W_q = np.asarray(W_q, np.float32).astype(bf)
    W_k = np.asarray(W_k, np.float32).astype(bf)
    W_v = np.asarray(W_v, np.float32).astype(bf)
    W_o = np.asarray(W_o, np.float32).astype(bf)
    b_o = np.asarray(b_o, np.float32).astype(bf).reshape(1, DIN)
    zeros_bo = np.zeros((1, DIN), bf)

    def lay_w(w, sl):   # [1024, 256] shard -> [128, 8*256]: t[p, c*256+n] = w[c*128+p, sl][n]
        return np.ascontiguousarray(
            w[:, sl].reshape(8, 128, QC).transpose(1, 0, 2).reshape(128, 8 * QC))

    def lay_wo(w, sl):  # [256, 1024] shard -> [128, 2*1024]
        return np.ascontiguousarray(
            w[sl, :].reshape(2, 128, DIN).transpose(1, 0, 2).reshape(128, 2 * DIN))

    in_maps = []
    for c in range(8):
        bi, g = c // 4, c % 4
        sl = slice(g * QC, (g + 1) * QC)
        in_maps.append({
            "x": xT[bi],
            "wq": lay_w(W_q, sl),
            "wk": lay_w(W_k, sl),
            "wv": lay_w(W_v, sl),
            "wo": lay_wo(W_o, sl),
            "bo": b_o if g == 0 else zeros_bo,
        })

    res = run_bass_kernel_spmd(nc, in_maps, list(range(8)), trace=TRACE)
    LAST_RESULTS = res
    outs = [np.asarray(r["out"], dtype=np.float32) for r in res.results]
    return np.stack([
        outs[0] + outs[1] + outs[2] + outs[3],
        outs[4] + outs[5] + outs[6] + outs[7],
    ])


if __name__ == "__main__":
    if "--compile-only" in sys.argv:
        import tempfile
        from concourse.bass_utils import compile_bass_kernel

        nc = build_nc()
        with tempfile.TemporaryDirectory() as td:
            print("walrus compiling...")
            neff = compile_bass_kernel(nc, td)
            print("COMPILE OK", neff)


# revision 29
# speedup vs baseline: 1.0044x; 1.0028x over previous
"""Trainium2 Bass kernel for nn_BaseAttention (causal MHA, b=2, n=2048, d=1024, 16 heads).

Sharding (8 cores): core c handles batch c//4 and heads 4*(c%4)..4*(c%4)+3.
- W_q/W_k/W_v column-sharded (256 cols/core), W_o row-sharded (256 rows/core).
- Each core computes a partial output [2048, 1024] in fp32; host sums the 4
  partials per batch (row-parallel out-projection) and stacks the 2 batches.

Per-core kernel (bf16 data path, fp32 PSUM accumulation; ~160 us HW exec,
rel err ~3.5e-3 vs the fp32 reference):
  - x is transposed + bf16-cast on the host (shared by the 4 cores of each
    batch) so x^T loads as full-bandwidth contiguous copy DMAs; weights are
    pre-laid-out to their SBUF layouts on the host as well.
  - Q^T/K^T projections emitted transposed; V natural with a ones column per
    head ([V|1] trick: the AV matmul yields ctx^T on psum partitions 0..63
    and the softmax row-sum at partition 64 in one pass).
  - attention per (head-pair, q-tile j): S^T = K_h @ Q_h^T on PE (even/odd
    heads on disjoint PE row-halves), exp on ACT over [128,1024] psum pairs
    with the 1/8 scale fused, causal mask via gpsimd affine_select
    (exp-then-zero; S/AV/mask all narrowed to the valid causal q-range), AV
    pipelined one i-pair behind S, normalization via DVE
    reciprocal_approx_fast (SBUF input only - broken from PSUM) + gpsimd
    partition_broadcast + DVE multiply.
  - projection work of round g+1 (or out-projection chunks in the last
    round) is woven between attention steps so the in-order PE queue never
    idles while ACT catches up (keeps HAM warm).
  - out-projection from ctx^T; bias added by a DVE tensor_add against a
    pre-broadcast b_o tile during the PSUM drain.
  - copy-DMAs and transpose-DMAs must not interleave (xbar mode transitions
    serialize); all DMAs here are copies. DMA issue is ~0.6us each on a
    sequencer, so transfers are consolidated into few instructions and
    split across the sync + scalar HWDGE queues.
"""
import sys, types

sys.path.insert(0, "/opt/trn_rl_repo")


def _install_ntff_shim():
    # antenv.axon_hooks is absent in this image; register the NTFF profile
    # hook via ctypes so run_bass_kernel_spmd(trace=True) works under axon.
    if "antenv.axon_hooks" in sys.modules:
        return
    try:
        sys.path.insert(0, "/root/.axon_site")
        from trn_agent_boot.trn_boot import _ntff_profile_via_ctypes

        hook = _ntff_profile_via_ctypes("/opt/axon/libaxon_pjrt.so")
        mod = types.ModuleType("antenv.axon_hooks")
        mod.get_axon_ntff_profile_hook = lambda: hook
        mod.set_axon_ntff_profile_hook = lambda h: None
        sys.modules["antenv.axon_hooks"] = mod
    except Exception:
        pass


_install_ntff_shim()

import numpy as np
import ml_dtypes
import concourse.bass as bass
import concourse.mybir as mybir
import concourse.tile as tile
from concourse import bacc
from concourse.bass_utils import run_bass_kernel_spmd
from contextlib import ExitStack

f32 = mybir.dt.float32
bf16 = mybir.dt.bfloat16
EXP = mybir.ActivationFunctionType.Exp

SEQ = 2048          # sequence length
DIN = 1024          # model dim (8 chunks of 128)
QC = 256            # q/k/v cols per core (4 heads x 64)
HD = 64             # head dim
NH = 4              # heads per core
NG = 4              # row groups of 512
VST = NH * 65       # Vones stride per row chunk (4 heads x (64 V + 1 ones))

TRACE = False
LAST_RESULTS = None


def build_nc():
    nc = bacc.Bacc()
    x_d = nc.dram_tensor("x", [DIN, SEQ], bf16, kind="ExternalInput")  # pre-transposed on host
    wq_d = nc.dram_tensor("wq", [128, 8 * QC], bf16, kind="ExternalInput")
    wk_d = nc.dram_tensor("wk", [128, 8 * QC], bf16, kind="ExternalInput")
    wv_d = nc.dram_tensor("wv", [128, 8 * QC], bf16, kind="ExternalInput")
    wo_d = nc.dram_tensor("wo", [128, 2 * DIN], bf16, kind="ExternalInput")
    bo_d = nc.dram_tensor("bo", [1, DIN], bf16, kind="ExternalInput")
    out_d = nc.dram_tensor("out", [SEQ, DIN], f32, kind="ExternalOutput")

    with tile.TileContext(nc, pool_alloc_mode="queue") as tc, ExitStack() as ctx:
        cst = ctx.enter_context(tc.tile_pool(name="cst", bufs=1))
        wr = ctx.enter_context(tc.tile_pool(name="wr", bufs=1))
        big = ctx.enter_context(tc.tile_pool(name="big", bufs=1))
        ptp = ctx.enter_context(tc.tile_pool(name="ptp", bufs=8))
        nrm = ctx.enter_context(tc.tile_pool(name="nrm", bufs=3))
        ob = ctx.enter_context(tc.tile_pool(name="ob", bufs=6))
        ps = ctx.enter_context(tc.tile_pool(name="ps", bufs=1, space="PSUM"))

        # ---- DMAs, ordered by first use (weights pre-laid-out on host).
        # weights issue on the scalar HWDGE queue, x on sync: parallel issue.
        def dma_w(wd, n_inner):
            name = wd.name + "_sb"
            t = wr.tile([128, 8 * n_inner], bf16, name=name)
            nc.scalar.dma_start(t[:], wd[:])
            return t

        # tiny bias DMA first so the gpsimd bias broadcast clears the FIFO
        # before any affine_select masks queue behind it
        bo_sb = cst.tile([1, DIN], bf16)
        nc.scalar.dma_start(bo_sb[:], bo_d[:])
        bo_f = cst.tile([1, DIN], f32)
        nc.vector.tensor_copy(bo_f[:], bo_sb[:])
        bias_bc = cst.tile([128, DIN], f32)
        nc.gpsimd.partition_broadcast(bias_bc[:], bo_f[:])

        # x arrives pre-transposed: straight contiguous copy DMAs at full BW.
        # xT[p, c*2048+r] = x[r, c*128+p] = x_d[c*128+p, r]
        wq_sb = wr.tile([128, 8 * QC], bf16, name="wq_sb")
        nc.scalar.dma_start(wq_sb[:, 0:2 * QC], wq_d[:, 0:2 * QC])
        nc.scalar.dma_start(wq_sb[:, 2 * QC:], wq_d[:, 2 * QC:])
        xT = big.tile([128, 8 * SEQ], bf16)
        xview = xT[:].rearrange("p (c r) -> p c r", r=SEQ)
        dview = x_d.rearrange("(c p) r -> p c r", p=128)
        nc.sync.dma_start(xview[:, 0:2, 0:512], dview[:, 0:2, 0:512])
        nc.sync.dma_start(xview[:, 2:8, 0:512], dview[:, 2:8, 0:512])
        wk_sb = dma_w(wk_d, QC)
        wv_sb = dma_w(wv_d, QC)
        for g in range(1, NG):
            nc.sync.dma_start(
                xview[:, :, g * 512:(g + 1) * 512],
                dview[:, :, g * 512:(g + 1) * 512],
            )
        wo_sb = cst.tile([128, 2 * DIN], bf16)
        nc.scalar.dma_start(wo_sb[:], wo_d[:])

        # ---- persistent activations ----
        qt_sb = [big.tile([128, SEQ], bf16, name=f"qt{t}") for t in range(2)]
        kt_sb = [big.tile([128, SEQ], bf16, name=f"kt{t}") for t in range(2)]
        vones = big.tile([128, 16 * VST], bf16)
        ctxt = [big.tile([128, SEQ], bf16, name=f"ctxt{t}") for t in range(2)]

        vview = vones.rearrange("p (r h e) -> p r h e", h=NH, e=65)
        nc.vector.memset(vview[:, :, :, 64], 1.0)

        # ---- emission helpers ----
        def emit_qk(g, t, wt, dst):
            prj = ps.tile([128, 512], f32, tag="b", bufs=2, name="prj")
            for c in range(8):
                nc.tensor.matmul(
                    prj[:],
                    wt[:, c * QC + t * 128: c * QC + t * 128 + 128],
                    xT[:, c * SEQ + g * 512: c * SEQ + g * 512 + 512],
                    start=(c == 0),
                    stop=(c == 7),
                )
            nc.vector.tensor_copy(dst[t][:, g * 512:(g + 1) * 512], prj[:])

        def emit_v(g, rc):
            rcg = 4 * g + rc
            vps = ps.tile([128, 256], f32, tag="b", bufs=2, name="vps")
            for c in range(8):
                nc.tensor.matmul(
                    vps[:],
                    xT[:, c * SEQ + rcg * 128: c * SEQ + rcg * 128 + 128],
                    wv_sb[:, c * QC:(c + 1) * QC],
                    start=(c == 0),
                    stop=(c == 7),
                )
            nc.vector.tensor_copy(
                vview[:, rcg, :, 0:64],
                vps[:].rearrange("p (h e) -> p h e", e=HD),
            )

        def emit_outproj(rc, n, tag="b"):
            ops = ps.tile([128, 512], f32, tag=tag, bufs=2, name="ops")
            for u in range(2):
                nc.tensor.matmul(
                    ops[:],
                    ctxt[u][:, rc * 128:(rc + 1) * 128],
                    wo_sb[:, u * DIN + n * 512: u * DIN + n * 512 + 512],
                    start=(u == 0),
                    stop=(u == 1),
                )
            osb = ob.tile([128, 512], f32, tag="o", name="osb")
            nc.vector.tensor_add(osb[:], ops[:], bias_bc[:, n * 512:(n + 1) * 512])
            nc.sync.dma_start(
                out_d[rc * 128:(rc + 1) * 128, n * 512:(n + 1) * 512], osb[:]
            )

        def proj_chunks(g):
            for t in range(2):
                yield lambda t=t: emit_qk(g, t, wq_sb, qt_sb)
                yield lambda t=t: emit_qk(g, t, wk_sb, kt_sb)
            for rc in range(4):
                yield lambda rc=rc: emit_v(g, rc)

        # ---- round 0 projections up-front ----
        for f in proj_chunks(0):
            f()

        # ---- main rounds: attention(j=g) woven with proj(g+1)/outproj ----
        for g in range(NG):
            j = g
            imax = 4 * j + 3
            npair = (imax + 1) // 2
            if g < NG - 1:
                filler = list(proj_chunks(g + 1))
            else:
                filler = [
                    (lambda rc=rc, n=n: emit_outproj(rc, n))
                    for rc in range(12)
                    for n in range(2)
                ]
            steps_total = 2 * npair
            fill_i = 0
            step = 0

            for u in range(2):           # head pair u: heads 2u, 2u+1
                avs = [ps.tile([65, 512], f32, tag="av", bufs=2, name=f"av{p}")
                       for p in range(2)]
                pts = [[], []]           # per parity: list of [128,1024] pair tiles
                for ip in range(npair):
                    i0 = 2 * ip
                    cur = []
                    for p in range(2):
                        sps = ps.tile([128, 1024], f32, tag="a", bufs=2, name="sps")
                        cur.append(sps)
                    for half in range(2):
                        i = i0 + half
                        off = max(0, 128 * i - 512 * j)
                        for p in range(2):
                            o = p * 64
                            nc.tensor.matmul(
                                cur[p][:, half * 512 + off:(half + 1) * 512],
                                kt_sb[u][o:o + 64, i * 128:(i + 1) * 128],
                                qt_sb[u][o:o + 64, j * 512 + off:(j + 1) * 512],
                                start=True,
                                stop=True,
                            )
                    for p in range(2):
                        pt = ptp.tile([128, 1024], bf16, tag="pt", name="pt")
                        nc.scalar.activation(pt[:], cur[p][:], EXP, scale=0.125)
                        for half in range(2):
                            i = i0 + half
                            if i >= 4 * j:
                                off = 128 * i - 512 * j
                                nc.gpsimd.affine_select(
                                    out=pt[:, half * 512 + off:(half + 1) * 512],
                                    in_=pt[:, half * 512 + off:(half + 1) * 512],
                                    compare_op=mybir.AluOpType.is_ge,
                                    fill=0.0,
                                    base=0,
                                    channel_multiplier=-1,
                                    pattern=[[1, 512 - off]],
                                )
                        pts[p].append(pt)
                    if ip >= 1:
                        kp = ip - 1
                        for p in range(2):
                            h = 2 * u + p
                            for half in range(2):
                                k = 2 * kp + half
                                off = max(0, 128 * k - 512 * j)
                                nc.tensor.matmul(
                                    avs[p][:, off:512],
                                    vones[:, k * VST + h * 65: k * VST + h * 65 + 65],
                                    pts[p][kp][:, half * 512 + off:(half + 1) * 512],
                                    start=(k == 0),
                                    stop=False,
                                )
                    step += 1
                    want = (len(filler) * step) // steps_total
                    while fill_i < want:
                        filler[fill_i]()
                        fill_i += 1
                # tail AVs for the last pair + immediate per-parity normalize
                kp = npair - 1
                for p in range(2):
                    h, o = 2 * u + p, p * 64
                    for half in range(2):
                        k = 2 * kp + half
                        off = max(0, 128 * k - 512 * j)
                        nc.tensor.matmul(
                            avs[p][:, off:512],
                            vones[:, k * VST + h * 65: k * VST + h * 65 + 65],
                            pts[p][kp][:, half * 512 + off:(half + 1) * 512],
                            start=(k == 0),
                            stop=(half == 1),
                        )
                    rsrow = nrm.tile([1, 512], f32, tag="rsrow", name="rsrow")
                    nc.vector.tensor_copy(rsrow[:], avs[p][64:65, :])
                    rinv = nrm.tile([1, 512], f32, tag="rinv", name="rinv")
                    nc.vector.reciprocal_approx_fast(rinv[:], rsrow[:])
                    bcast = nrm.tile([64, 512], f32, tag="bcast", name="bcast")
                    nc.gpsimd.partition_broadcast(bcast[:], rinv[:])
                    nc.vector.tensor_mul(
                        ctxt[u][o:o + 64, j * 512:(j + 1) * 512],
                        avs[p][0:64, :],
                        bcast[:],
                    )
            while fill_i < len(filler):
                filler[fill_i]()
                fill_i += 1

        # ---- final out-projection chunks (use idle av psum slots too) ----
        for rc in range(12, 16):
            for n in range(2):
                emit_outproj(rc, n, tag="av" if (rc + n) % 2 else "b")

    nc.compile()
    return nc


_NC = None


def _get_nc():
    global _NC
    if _NC is None:
        _NC = build_nc()
    return _NC


def kernel(x, W_q, W_k, W_v, W_o, b_o):
    global LAST_RESULTS
    nc = _get_nc()
    bf = ml_dtypes.bfloat16
    x = np.asarray(x, np.float32).astype(bf)
    # pre-transpose per batch (shared by the 4 cores of each batch)
    xT = [np.ascontiguousarray(x[bi].T) for bi in range(2)]
    W_q = np.asarray(W_q, np.float32).astype(bf)
    W_k = np.asarray(W_k, np.float32).astype(bf)
    W_v = np.asarray(W_v, np.float32).astype(bf)
    W_o = np.asarray(W_o, np.float32).astype(bf)
    b_o = np.asarray(b_o, np.float32).astype(bf).reshape(1, DIN)
    zeros_bo = np.zeros((1, DIN), bf)

    def lay_w(w, sl):   # [1024, 256] shard -> [128, 8*256]: t[p, c*256+n] = w[c*128+p, sl][n]
        return np.ascontiguousarray(
            w[:, sl].reshape(8, 128, QC).transpose(1, 0, 2).reshape(128, 8 * QC))

    def lay_wo(w, sl):  # [256, 1024] shard -> [128, 2*1024]
        return np.ascontiguousarray(
            w[sl, :].reshape(2, 128, DIN).transpose(1, 0, 2).reshape(128, 2 * DIN))

    in_maps = []
    for c in range(8):
        bi, g = c // 4, c % 4
        sl = slice(g * QC, (g + 1) * QC)
        in_maps.append({
            "x": xT[bi],
            "wq": lay_w(W_q, sl),
            "wk": lay_w(W_k, sl),
            "wv": lay_w(W_v, sl),
            "wo": lay_wo(W_o, sl),
            "bo": b_o if g == 0 else zeros_bo,
        })

    res = run_bass_kernel_spmd(nc, in_maps, list(range(8)), trace=TRACE)
    LAST_RESULTS = res
    outs = [np.asarray(r["out"], dtype=np.float32) for r in res.results]
    return np.stack([
        outs[0] + outs[1] + outs[2] + outs[3],
        outs[4] + outs[5] + outs[6] + outs[7],
    ])


if __name__ == "__main__":
    if "--compile-only" in sys.argv:
        import tempfile
        from concourse.bass_utils import compile_bass_kernel

        nc = build_nc()
        with tempfile.TemporaryDirectory() as td:
            print("walrus compiling...")
            neff = compile_bass_kernel(nc, td)
            print("COMPILE OK", neff)
